# revision 26
# baseline (speedup 1.0000x reference)
"""Trainium2 Bass kernel for MoE-with-LoRA-experts (nn_MoE_64098091925598).

Reference computation (N=8192 tokens, D=1024, E=8 experts, R=16, top-2):
    logits  = x @ W_gate.T                      [N, E]
    combine = scatter(softmax(top2(logits)))    [N, E] (2 nonzeros/row)
    moe     = sum_e combine[:,e] * (x @ A_e @ B_e)
    out     = moe + x @ W_base.T + b_base

Strategy: data-parallel over tokens across 8 NeuronCores (1024 tokens per
core); every core computes all 8 LoRA experts densely and masks by the
combine weights (expert FLOPs are tiny vs the base linear). All matmuls
in float32r (full PE rate at moving>=256). Gating stays fp32 — bf16
logits flip ~14/8192 top-2 picks and blow the error budget.

Key structural points (from trace analysis of the 77.6us baseline):
- Packed DMAs: host lays x / W_base out as [128p][c][n] so each logical
  load is one DMA with 4-8KB-per-partition descriptors (the baseline's
  52 small loads serialized ~620ns DGE descriptor-gen per [128,512]
  tile on the issuing engines).
- Emission order interleaves the gating/LoRA front-end with the main
  accumulation loop per 512-token wave so the PE never sits idle:
  FE(w0) -> main(dt0, j0..3) -> FE(w1) -> main(dt0, j4..7) -> main(dt1).
- 512-col warmup matmuls: the HAM power manager ungates the PE array
  (k=4 half clock -> k=8) only after a sustained high-activity window,
  and any multi-us PE idle gap re-gates it. The warmup bridges engine
  start (~7us) to the first data arrival so the stream stays dense.
- Queue plan: the sync HWDGE ring starts ~2.7us before scalar (which
  pays the ACT_TABLE_LOAD) and sustains 2-3x its bandwidth, so sync
  carries all of x wave-0 and the 6/8 front of each later tensor.
- Output is written as bf16 (2MB instead of 4MB; host upconverts) and
  the LoRA A/expand/combine tensors ride in bf16, well inside the 2e-2
  error budget. W_base/x stay f32r: the gating top-2 selection needs
  full-precision logits (bf16 flips ~14/8192 picks -> rel err 0.28),
  and bf16 W_base tiles hit a hardware corruption (even-column garbage
  in the first dout half) that CoreSim does not reproduce, so the base
  matmul stays f32r at the same PE rate.
"""

import numpy as np
import ml_dtypes

import concourse.mybir as mybir
import concourse.tile as tile
from concourse import bacc
from concourse.bass_utils import run_bass_kernel_spmd

N_TOK, D, E, R, TOPK = 8192, 1024, 8, 16, 2
CORES = 8
NS = N_TOK // CORES  # tokens per core
ER = E * R  # 128, stacked expert-rank dim
DC = D // 128  # 8 contraction chunks
NJ = NS // 128  # 8 token chunks per core
NT = NS // 512  # 2 wide token tiles (waves) per core
JT = NJ // NT  # 4 token chunks per wave
DT = D // 512  # 2 dout tiles

f32 = mybir.dt.float32
f32r = mybir.dt.float32r
bf16 = mybir.dt.bfloat16

N_WARM = 20  # wide dummy matmuls bridging PE start (~7us) to first data
# 512-col dummies at high PE utilization: the HAM power manager ungates
# the array (k=4 -> k=8) only after a sustained high-activity window; the
# old 64-col warmup left the whole front-end throttled to half clock.

_CACHE: dict = {}


def _emit_front_end(nc, tc, pools, t, xt, xtb, wg_sb, a_sb, exp_sb, HT_sb, ident):
    """Gating + LoRA-h for one 512-token wave t; writes HT_sb[:, wave]."""
    pg, pmt, ps_tp, ps_mm = pools
    sl = slice(t * 512, (t + 1) * 512)

    # bf16 copies of this wave's x chunks on the Activation engine (the
    # hardware rejects mixed f32r x bf16 matmuls, so the bf16 main loop
    # needs a bf16 x). The gating logits keep the full-precision f32r x.
    for c in range(DC):
        nc.scalar.activation(
            xtb[t][:, c, :],
            xt[t][:, c, :].bitcast(f32),
            mybir.ActivationFunctionType.Copy,
        )

    # logits^T [E, 512] for this wave
    lgT_ps = ps_mm.tile([E, 512], f32, tag="mm")
    for c in range(DC):
        nc.tensor.matmul(
            lgT_ps, wg_sb[:, c, :], xt[t][:, c, :], start=(c == 0), stop=(c == DC - 1)
        )
    # LoRA h^T = A_flat^T @ x^T, emitted right after so the PE stays busy
    # while the vector engine runs the top-k chain.
    h_ps = ps_mm.tile([ER, 512], f32, tag="mm")
    for c in range(DC):
        nc.tensor.matmul(
            h_ps, a_sb[:, c, :], xtb[t][:, c, :], start=(c == 0), stop=(c == DC - 1)
        )

    lgT_sb = pg.tile([E, 512], f32, tag=f"lgT{t}")
    nc.vector.tensor_copy(lgT_sb, lgT_ps)

    # token-major logits chunks + sorted top-8 per token
    lg3 = pg.tile([128, JT, E], f32, tag=f"lg3_{t}")
    mx = pg.tile([128, JT, E], f32, tag=f"mx{t}")
    for r in range(JT):
        tr_ps = ps_tp.tile([128, E], f32, tag="tp")
        nc.tensor.transpose(
            tr_ps, lgT_sb[:, r * 128 : (r + 1) * 128], ident[0:E, 0:E]
        )
        nc.vector.tensor_copy(lg3[:, r, :], tr_ps)
        nc.vector.max(out=mx[:, r, :], in_=lg3[:, r, :])

    # combine = 1{l==v1}*sigmoid(v1-v2) + 1{l==v2}*sigmoid(v2-v1)
    v1 = mx[:, :, 0:1]
    v2 = mx[:, :, 1:2]
    d21 = pg.tile([128, JT, 1], f32, tag=f"d21_{t}")
    nc.vector.tensor_sub(d21, v2, v1)
    w1 = pg.tile([128, JT, 1], f32, tag=f"w1_{t}")
    w2 = pg.tile([128, JT, 1], f32, tag=f"w2_{t}")
    nc.scalar.activation(w2, d21, mybir.ActivationFunctionType.Sigmoid)
    nc.scalar.activation(w1, d21, mybir.ActivationFunctionType.Sigmoid, scale=-1.0)

    eq1 = pg.tile([128, JT, E], f32, tag=f"eq1_{t}")
    eq2 = pg.tile([128, JT, E], f32, tag=f"eq2_{t}")
    cb = pg.tile([128, JT, E], f32, tag=f"cb{t}")
    bs = [128, JT, E]
    nc.vector.tensor_tensor(eq1, lg3, v1.to_broadcast(bs), mybir.AluOpType.is_equal)
    nc.vector.tensor_tensor(eq2, lg3, v2.to_broadcast(bs), mybir.AluOpType.is_equal)
    nc.vector.tensor_tensor(eq1, eq1, w1.to_broadcast(bs), mybir.AluOpType.mult)
    nc.vector.tensor_tensor(eq2, eq2, w2.to_broadcast(bs), mybir.AluOpType.mult)
    nc.vector.tensor_add(cb, eq1, eq2)

    # combine^T [E, 512] via PE transpose per chunk
    cT_sb = pg.tile([E, 512], bf16, tag=f"cT{t}")
    for r in range(JT):
        cT_ps = ps_tp.tile([E, 128], f32, tag="tp")
        nc.tensor.transpose(cT_ps, cb[:, r, :], ident)
        nc.vector.tensor_copy(cT_sb[:, r * 128 : (r + 1) * 128], cT_ps)

    # H^T = h^T * expand(combine^T)
    h_sb = pmt.tile([ER, 512], f32, tag="hsb")
    nc.vector.tensor_copy(h_sb, h_ps)
    ce_ps = ps_mm.tile([ER, 512], f32, tag="mm")
    nc.tensor.matmul(ce_ps, exp_sb, cT_sb, start=True, stop=True)
    nc.vector.tensor_tensor(HT_sb[:, sl], ce_ps, h_sb, mybir.AluOpType.mult)


def _kernel_body(nc, tc, dram):
    xw0, xw1, wb0, wb1, a3, wg3, b2, expm, b_vec, ident_hbm, out = dram
    xw = (xw0, xw1)
    wbw = (wb0, wb1)

    from contextlib import ExitStack

    ctx = ExitStack()
    pw = ctx.enter_context(tc.tile_pool(name="weights", bufs=1))
    pg = ctx.enter_context(tc.tile_pool(name="gating", bufs=1))
    pmt = ctx.enter_context(tc.tile_pool(name="mmtmp", bufs=2))
    pout = ctx.enter_context(tc.tile_pool(name="outsb", bufs=4))
    ps_tp = ctx.enter_context(tc.tile_pool(name="ps_tp", bufs=2, space="PSUM"))
    ps_mm = ctx.enter_context(tc.tile_pool(name="ps_mm", bufs=2, space="PSUM"))
    ps_out = ctx.enter_context(tc.tile_pool(name="ps_out", bufs=4, space="PSUM"))

    # ---- PE prewarm: garbage matmuls, no data deps, never read -----
    warm_sb = pw.tile([128, 512], f32r, tag="warm")
    warm_ps = ps_mm.tile([128, 512], f32, tag="mm")
    nc.vector.memset(warm_sb.bitcast(f32), 0.0)
    for _ in range(N_WARM):
        nc.tensor.matmul(
            warm_ps, warm_sb[:, 0:128], warm_sb, start=True, stop=True
        )

    # ---- Load phase: two HWDGE rings, priority order ----------------
    # Every large tensor is split half/half across both rings so each
    # dependency completes as early as possible. Priority:
    #   [x w0 | smalls] -> wb dt0 -> b_flat -> x w1 -> wb dt1
    xt = [
        pw.tile([128, DC, 512], f32r, tag=f"xt{t}", name=f"xt{t}") for t in range(NT)
    ]
    xtb = [
        pw.tile([128, DC, 512], bf16, tag=f"xtb{t}", name=f"xtb{t}")
        for t in range(NT)
    ]
    wbt = [
        pw.tile([128, DC, 512], f32r, tag=f"wbt{d}", name=f"wbt{d}") for d in range(DT)
    ]
    wg_sb = pw.tile([128, DC, E], f32r, tag="wg")
    a_sb = pw.tile([128, DC, ER], bf16, tag="a")
    exp_sb = pw.tile([E, ER], bf16, tag="expand")
    b_sb = pw.tile([ER, D], f32r, tag="bflat")
    bias_sb = pw.tile([128, D], f32, tag="bias")

    ident = pw.tile([128, 128], f32, tag="ident")

    H = 5  # sync ring carries 5 of 8 chunks: it starts ~1.3us earlier
    # than scalar (which pays the ACT_TABLE_LOAD) and drains faster
    nc.sync.dma_start(out=xt[0][:, 0:H, :], in_=xw[0][:, 0:H, :])
    nc.sync.dma_start(out=a_sb, in_=a3)
    nc.sync.dma_start(out=wbt[0][:, 0:H, :], in_=wbw[0][:, 0:H, :])
    nc.sync.dma_start(out=bias_sb, in_=b_vec.to_broadcast([128, D]))
    nc.sync.dma_start(out=b_sb, in_=b2)
    nc.sync.dma_start(out=xt[1][:, 0:H, :], in_=xw[1][:, 0:H, :])
    nc.sync.dma_start(out=wbt[1][:, 0:H, :], in_=wbw[1][:, 0:H, :])
    # ring B = scalar
    nc.scalar.dma_start(out=wg_sb, in_=wg3)
    nc.scalar.dma_start(out=xt[0][:, H:DC, :], in_=xw[0][:, H:DC, :])
    nc.scalar.dma_start(out=exp_sb, in_=expm)
    nc.scalar.dma_start(out=ident, in_=ident_hbm)
    nc.scalar.dma_start(out=wbt[0][:, H:DC, :], in_=wbw[0][:, H:DC, :])
    nc.scalar.dma_start(out=xt[1][:, H:DC, :], in_=xw[1][:, H:DC, :])
    nc.scalar.dma_start(out=wbt[1][:, H:DC, :], in_=wbw[1][:, H:DC, :])

    HT_sb = pg.tile([ER, NS], f32r, tag="HT")
    pools = (pg, pmt, ps_tp, ps_mm)

    def main_tile(dt, j):
        dsl = slice(dt * 512, (dt + 1) * 512)
        jsl = slice(j * 128, (j + 1) * 128)
        jh, jr = divmod(j, JT)
        out_ps = ps_out.tile([128, 512], f32, tag="out")
        for c in range(DC):
            nc.tensor.matmul(
                out_ps,
                xt[jh][:, c, jr * 128 : (jr + 1) * 128],
                wbt[dt][:, c, :],
                start=(c == 0),
                stop=False,
            )
        nc.tensor.matmul(out_ps, HT_sb[:, jsl], b_sb[:, dsl], start=False, stop=True)
        out_sb = pout.tile([128, 512], bf16, tag="osb")
        nc.vector.tensor_add(out_sb, out_ps, bias_sb[:, dsl])
        eng = nc.sync if (j + dt) % 2 == 0 else nc.scalar
        eng.dma_start(out=out[jsl, dsl], in_=out_sb)

    # ---- Interleaved schedule: FE(w) then main tiles of that wave ----
    _emit_front_end(nc, tc, pools, 0, xt, xtb, wg_sb, a_sb, exp_sb, HT_sb, ident)
    for j in range(0, JT):
        main_tile(0, j)
    _emit_front_end(nc, tc, pools, 1, xt, xtb, wg_sb, a_sb, exp_sb, HT_sb, ident)
    for j in range(JT, NJ):
        main_tile(0, j)
    for j in range(NJ):
        main_tile(1, j)

    ctx.close()


def build_nc():
    nc = bacc.Bacc(
        "TRN2",
        target_bir_lowering=False,
        debug=False,
        enable_asserts=False,
        num_devices=CORES,
    )
    xw0 = nc.dram_tensor("xw0", [128, DC, 512], f32, kind="ExternalInput").ap()
    xw1 = nc.dram_tensor("xw1", [128, DC, 512], f32, kind="ExternalInput").ap()
    wb0 = nc.dram_tensor("wb0", [128, DC, 512], f32, kind="ExternalInput").ap()
    wb1 = nc.dram_tensor("wb1", [128, DC, 512], f32, kind="ExternalInput").ap()
    a3 = nc.dram_tensor("a3", [128, DC, ER], bf16, kind="ExternalInput").ap()
    wg3 = nc.dram_tensor("wg3", [128, DC, E], f32, kind="ExternalInput").ap()
    b2 = nc.dram_tensor("b2", [ER, D], f32, kind="ExternalInput").ap()
    expm = nc.dram_tensor("expm", [E, ER], bf16, kind="ExternalInput").ap()
    b_vec = nc.dram_tensor("b_vec", [1, D], f32, kind="ExternalInput").ap()
    ident_h = nc.dram_tensor("ident_h", [128, 128], f32, kind="ExternalInput").ap()
    out = nc.dram_tensor("out", [NS, D], bf16, kind="ExternalOutput").ap()

    dram = (
        xw0.bitcast(f32r),
        xw1.bitcast(f32r),
        wb0.bitcast(f32r),
        wb1.bitcast(f32r),
        a3,
        wg3.bitcast(f32r),
        b2.bitcast(f32r),
        expm,
        b_vec,
        ident_h,
        out,
    )
    with tile.TileContext(nc) as tc:
        _kernel_body(nc, tc, dram)
    nc.compile()
    return nc


def host_prep(x, W_gate, A, B, W_base, b_base):
    """Shard + lay out the full inputs into 8 per-core input maps."""
    # shared (replicated) tensors, packed [128p][c][...]
    bfl = ml_dtypes.bfloat16
    wbT = np.ascontiguousarray(W_base.T)  # [din, dout]
    wbr = wbT.reshape(DC, 128, D).transpose(1, 0, 2)  # [128, DC, D]
    wb0 = np.ascontiguousarray(wbr[:, :, 0:512])
    wb1 = np.ascontiguousarray(wbr[:, :, 512:1024])
    a3 = np.ascontiguousarray(
        A.transpose(1, 0, 2).reshape(D, ER).reshape(DC, 128, ER).transpose(1, 0, 2)
    ).astype(bfl)
    wg3 = np.ascontiguousarray(
        W_gate.T.reshape(DC, 128, E).transpose(1, 0, 2)
    )
    b2 = np.ascontiguousarray(B.reshape(ER, D))
    expm = np.zeros((E, ER), dtype=bfl)
    for e in range(E):
        expm[e, e * R : (e + 1) * R] = 1.0
    b_vec = np.ascontiguousarray(b_base.reshape(1, D))
    ident_h = np.eye(128, dtype=np.float32)

    in_maps = []
    for c in range(CORES):
        xc = x[c * NS : (c + 1) * NS]  # [NS, D]
        xr = xc.T.reshape(DC, 128, NS).transpose(1, 0, 2)  # [128, DC, NS]
        in_maps.append(
            {
                "xw0": np.ascontiguousarray(xr[:, :, 0:512]),
                "xw1": np.ascontiguousarray(xr[:, :, 512:1024]),
                "wb0": wb0,
                "wb1": wb1,
                "a3": a3,
                "wg3": wg3,
                "b2": b2,
                "expm": expm,
                "b_vec": b_vec,
                "ident_h": ident_h,
            }
        )
    return in_maps


def kernel(x, W_gate, A, B, W_base, b_base):
    x = np.asarray(x, dtype=np.float32)
    W_gate = np.asarray(W_gate, dtype=np.float32)
    A = np.asarray(A, dtype=np.float32)
    B = np.asarray(B, dtype=np.float32)
    W_base = np.asarray(W_base, dtype=np.float32)
    b_base = np.asarray(b_base, dtype=np.float32)

    if "nc" not in _CACHE:
        _CACHE["nc"] = build_nc()
    nc = _CACHE["nc"]

    in_maps = host_prep(x, W_gate, A, B, W_base, b_base)
    res = run_bass_kernel_spmd(nc, in_maps, core_ids=list(range(CORES)))
    return np.concatenate(
        [np.asarray(res.results[c]["out"]).astype(np.float32) for c in range(CORES)],
        axis=0,
    )
